# revision 1
# baseline (speedup 1.0000x reference)
"""BERT-base encoder (12 layers, B=8 S=512 H=768) on 8 Trainium2 NeuronCores.

Strategy: data-parallel over batch — each core runs the full 12-layer
encoder for one sequence, weights replicated, no collectives.

On-chip layout: activations are kept feature-major ("xT": [768 feats ->
6x128 partitions, 512 tokens free]) so row-major weight matrices serve
directly as the matmul stationary operand (lhsT) and GEMM outputs stay
feature-major. Matmuls run in float32r (full-speed PE, fp32 PSUM accum).

Attention per head: logits are computed transposed ([k_tok, q], contraction
over D=64; adjacent heads pack into disjoint PE row groups), softmax runs as
Exp(0.125*logits) on the scalar engine (logits are O(1) so no max-subtraction
is needed), and the key-padding mask is applied by zeroing masked tokens'
rows of the token-major V tile — including its appended ones-column, so the
softmax denominator (produced by the same matmul that computes attn@V via
that ones-column) also excludes masked keys, exactly like exp(-1e4) -> 0.

LayerNorm runs feature-major: partition sums via ones-matmuls, mean/rstd
broadcast back across partitions via rank-1 matmuls.
"""

import numpy as np

import concourse.bass as bass
import concourse.mybir as mybir
from concourse.tile import TileContext
from concourse.vector_clock import ScopedClock
from concourse.masks import make_identity

F32 = mybir.dt.float32
F32R = mybir.dt.float32r
I32 = mybir.dt.int32
AF = mybir.ActivationFunctionType
OP = mybir.AluOpType

B, S, H, L, NH, FF, D = 8, 512, 768, 12, 12, 3072, 64
V_VOCAB, T_VOCAB = 30522, 2
KC = H // 128          # 6 feature chunks
FC = FF // 128         # 24 ffn chunks
TC = S // 128          # 4 token chunks
EPS = 1e-12
N_CORES = 8


# --- walrus workarounds -----------------------------------------------------
# 1) This walrus build allows only one sync-wait command per instruction for
#    several ISA structs; split extra waits onto NoOps (same engine, just
#    before the instruction — engines execute their stream in order).
import json as _json

_WAIT_LIMITS = {}
_DEF_LIMIT = 1
_wcount = [0]


def _fix_block(block):
    insts = block.get("instructions")
    if insts:
        out = []
        for ins in insts:
            si = ins.get("sync_info")
            waits = (si or {}).get("on_wait") or []
            limit = _WAIT_LIMITS.get(ins.get("opcode"), _DEF_LIMIT)
            if len(waits) > limit:
                keep = waits[: max(0, limit - 1)] if limit > 1 else []
                move = waits[len(keep):-1]
                last = [waits[-1]]
                for w in move:
                    _wcount[0] += 1
                    out.append({
                        "name": f"I-wsplit-{_wcount[0]}",
                        "opcode": "NoOp",
                        "engine": ins.get("engine"),
                        "ins": [],
                        "outs": [],
                        "debug": ins.get("debug"),
                        "sync_info": {"on_wait": [w], "on_update": []},
                    })
                si["on_wait"] = keep + last
            out.append(ins)
        block["instructions"] = out
    for sub in block.get("blocks", []) or []:
        _fix_block(sub)


def _fix_module_json(data: bytes) -> bytes:
    d = _json.loads(data)
    for fn in d.get("functions", []):
        for b in fn.get("blocks", []) or []:
            _fix_block(b)
    return _json.dumps(d).encode()


_patched = [False]


def _install_waitfix():
    if _patched[0]:
        return
    _patched[0] = True
    orig = bass.Bass.to_json_bytes

    def patched(self):
        return _fix_module_json(orig(self))

    bass.Bass.to_json_bytes = patched


# 2) The Tile kernel-tail drain carries one wait per live semaphore; split
#    them the same way at IR build time.
class PTC(TileContext):
    def _drain_and_barrier(self, tick_clock, wait_clock):
        drain_inst = self.nc.sync.drain()
        wait_clock.add_sem_waits(
            drain_inst.ins, ScopedClock({None: tick_clock.global_clock})
        )
        si = drain_inst.ins.sync_info
        waits = list(si.on_wait or [])
        if len(waits) > 1:
            si.on_wait = waits[:1]
            for w in waits[1:]:
                nop = self.nc.sync.nop(nofuse=True, hint="tail_wait_split")
                nop.ins.sync_info = mybir.SyncInfo(on_wait=[w], on_update=[])
        self.nc.all_engine_barrier()
        popped = self.nc._tile_sem_poison_stack.pop()
        assert popped is self._sem_poison
        self.nc.clear_and_free_semaphores(list(self.sems.allocated().values()))
        self.nc.all_engine_barrier()


# --- kernel builder ---------------------------------------------------------

def build_nc(n_layers=L):
    nc = bass.Bass()

    ids_d = nc.dram_tensor("input_ids", [S], I32, kind="ExternalInput")
    tids_d = nc.dram_tensor("type_ids", [S], I32, kind="ExternalInput")
    wemb_d = nc.dram_tensor("word_emb", [V_VOCAB, H], F32, kind="ExternalInput")
    pemb_d = nc.dram_tensor("pos_emb", [S, H], F32, kind="ExternalInput")
    temb_d = nc.dram_tensor("type_emb", [T_VOCAB, H], F32, kind="ExternalInput")
    embs_d = nc.dram_tensor("emb_ln_scale", [H], F32, kind="ExternalInput")
    embb_d = nc.dram_tensor("emb_ln_bias", [H], F32, kind="ExternalInput")
    wq_d = nc.dram_tensor("wq", [L, H, H], F32, kind="ExternalInput")
    bq_d = nc.dram_tensor("bq", [L, H], F32, kind="ExternalInput")
    wk_d = nc.dram_tensor("wk", [L, H, H], F32, kind="ExternalInput")
    bk_d = nc.dram_tensor("bk", [L, H], F32, kind="ExternalInput")
    wv_d = nc.dram_tensor("wv", [L, H, H], F32, kind="ExternalInput")
    bv_d = nc.dram_tensor("bv", [L, H], F32, kind="ExternalInput")
    wo_d = nc.dram_tensor("wo", [L, H, H], F32, kind="ExternalInput")
    bo_d = nc.dram_tensor("bo", [L, H], F32, kind="ExternalInput")
    l1s_d = nc.dram_tensor("ln1_scale", [L, H], F32, kind="ExternalInput")
    l1b_d = nc.dram_tensor("ln1_bias", [L, H], F32, kind="ExternalInput")
    w1_d = nc.dram_tensor("w1", [L, H, FF], F32, kind="ExternalInput")
    b1_d = nc.dram_tensor("b1", [L, FF], F32, kind="ExternalInput")
    w2_d = nc.dram_tensor("w2", [L, FF, H], F32, kind="ExternalInput")
    b2_d = nc.dram_tensor("b2", [L, H], F32, kind="ExternalInput")
    l2s_d = nc.dram_tensor("ln2_scale", [L, H], F32, kind="ExternalInput")
    l2b_d = nc.dram_tensor("ln2_bias", [L, H], F32, kind="ExternalInput")
    out_d = nc.dram_tensor("out", [S, H], F32, kind="ExternalOutput")

    with PTC(nc) as tc:
        with (
            tc.tile_pool(name="const", bufs=1) as cpool,
            tc.tile_pool(name="stream", bufs=5) as spool,
            tc.tile_pool(name="exp", bufs=2) as epool,
            tc.tile_pool(name="wc", bufs=6) as wcpool,
            tc.tile_pool(name="wr", bufs=4) as wrpool,
            tc.tile_pool(name="tmp", bufs=3) as tpool,
            tc.tile_pool(name="gel", bufs=4) as gpool,
            tc.tile_pool(name="rows", bufs=4) as rpool,
            tc.tile_pool(name="par", bufs=2) as ppool,
        ):
            # ---- constants -------------------------------------------------
            ident = cpool.tile([128, 128], F32)
            make_identity(nc, ident[:])
            ones_f = cpool.tile([128, 128], F32)
            nc.gpsimd.memset(ones_f[:], 1.0)
            ones_row = cpool.tile([1, 128], F32R)   # lhsT for partition bcast
            nc.vector.tensor_copy(ones_row[:], ones_f[:1, :])
            ones_col = cpool.tile([128, 1], F32R)   # lhsT for partition sums
            nc.vector.tensor_copy(ones_col[:], ones_f[:, :1])

            eps_t = cpool.tile([1, 1], F32)
            nc.vector.memset(eps_t[:], EPS)

            ids_t = cpool.tile([128, TC], I32)
            nc.sync.dma_start(ids_t[:], ids_d[:].rearrange("(t p) -> p t", p=128))
            tids_t = cpool.tile([128, TC], I32)
            nc.sync.dma_start(tids_t[:], tids_d[:].rearrange("(t p) -> p t", p=128))

            ids_f = cpool.tile([128, TC], F32)
            nc.vector.tensor_copy(ids_f[:], ids_t[:])
            # zmask[p, t] = 0.0 where token id == 0 (padding), else 1.0
            zmask = cpool.tile([128, TC], F32)
            nc.vector.tensor_scalar(zmask[:], ids_f[:], 0.0, -1.0,
                                    OP.is_equal, OP.mult)
            nc.vector.tensor_scalar(zmask[:], zmask[:], 1.0, None, OP.add)

            # token-major V with a ones column per head (65-wide per head)
            v_aug = cpool.tile([128, TC, NH * 65], F32R)
            nc.vector.tensor_copy(
                v_aug[:].rearrange("p t (h c) -> p t h c", c=65)[:, :, :, 64:65],
                ones_f[:, :1].to_broadcast([128, TC, NH, 1]),
            )

            # ---- embedding (token-major), then transpose to feature-major --
            eT = spool.tile([128, KC, S], F32R, tag="s6")
            with (
                tc.tile_pool(name="embp", bufs=2) as embp,
                tc.tile_pool(name="embps", bufs=4, space="PSUM") as embps,
            ):
                # broadcast of type_emb rows replaced by per-token gather
                for t in range(TC):
                    wg = embp.tile([128, H], F32, tag="eg", bufs=2)
                    nc.gpsimd.indirect_dma_start(
                        out=wg[:], out_offset=None, in_=wemb_d[:],
                        in_offset=bass.IndirectOffsetOnAxis(ap=ids_t[:, t:t + 1], axis=0),
                    )
                    tg = embp.tile([128, H], F32, tag="eg2", bufs=1)
                    nc.gpsimd.indirect_dma_start(
                        out=tg[:], out_offset=None, in_=temb_d[:],
                        in_offset=bass.IndirectOffsetOnAxis(ap=tids_t[:, t:t + 1], axis=0),
                    )
                    pg = embp.tile([128, H], F32, tag="eg3", bufs=1)
                    nc.sync.dma_start(pg[:], pemb_d[128 * t:128 * (t + 1), :])
                    et = embp.tile([128, H], F32, tag="et", bufs=1)
                    nc.vector.tensor_tensor(et[:], wg[:], tg[:], op=OP.add)
                    nc.vector.tensor_tensor(et[:], et[:], pg[:], op=OP.add)
                    for f in range(KC):
                        tp = embps.tile([128, 128], F32)
                        nc.tensor.transpose(tp[:], et[:, 128 * f:128 * (f + 1)], ident[:])
                        nc.vector.tensor_copy(eT[:, f, 128 * t:128 * (t + 1)], tp[:])

            # embedding layernorm
            es_col = ppool.tile([128, KC], F32, tag="pc6", bufs=18)
            nc.sync.dma_start(es_col[:], embs_d[:].rearrange("(k p) -> p k", p=128))
            eb_col = ppool.tile([128, KC], F32, tag="pc6", bufs=18)
            nc.sync.dma_start(eb_col[:], embb_d[:].rearrange("(k p) -> p k", p=128))
            hT = _layer_norm(nc, tc, spool, tpool, rpool, eT, es_col, eb_col,
                             ones_col, ones_row, eps_t, F32R)

            for l in range(n_layers):
                hT = _encoder_layer(
                    nc, tc, l, hT,
                    spool, epool, wcpool, wrpool, tpool, gpool, rpool, ppool,
                    v_aug, zmask, ones_col, ones_row, eps_t,
                    wq_d, bq_d, wk_d, bk_d, wv_d, bv_d, wo_d, bo_d,
                    l1s_d, l1b_d, w1_d, b1_d, w2_d, b2_d, l2s_d, l2b_d,
                    last=(l == n_layers - 1),
                )

            # ---- final transpose back to token-major + store ---------------
            with tc.tile_pool(name="finps", bufs=4, space="PSUM") as finps:
                for t in range(TC):
                    ot = tpool.tile([128, H], F32, tag="fin", bufs=2)
                    for f in range(KC):
                        tp = finps.tile([128, 128], F32)
                        nc.tensor.transpose(
                            tp[:], hT[:, f, 128 * t:128 * (t + 1)], ident[:]
                        )
                        nc.vector.tensor_copy(ot[:, 128 * f:128 * (f + 1)], tp[:])
                    nc.sync.dma_start(out_d[128 * t:128 * (t + 1), :], ot[:])

    return nc


def _layer_norm(nc, tc, spool, tpool, rpool, x, scale_col, bias_col,
                ones_col, ones_row, eps_t, out_dtype, psum_pool=None):
    """x: [128, KC, S] f32r feature-major. Returns normalized stream tile."""
    sq = spool.tile([128, KC, S], F32R, tag="s6")
    for k in range(KC):
        nc.scalar.activation(sq[:, k], x[:, k], AF.Square)

    import contextlib
    pool_cm = (tc.tile_pool(name="lnps", bufs=1, space="PSUM")
               if psum_pool is None else contextlib.nullcontext(psum_pool))
    with pool_cm as lnps:
        ps1 = lnps.tile([1, S], F32, tag="st", bufs=1, name="ln_s1")
        ps2 = lnps.tile([1, S], F32, tag="st2", bufs=1, name="ln_s2")
        for k in range(KC):
            nc.tensor.matmul(ps1[:], ones_col[:], x[:, k],
                             start=(k == 0), stop=(k == KC - 1))
        for k in range(KC):
            nc.tensor.matmul(ps2[:], ones_col[:], sq[:, k],
                             start=(k == 0), stop=(k == KC - 1))

        # mean: broadcast early so the (x - mu) chunk ops overlap the
        # rstd row chain
        mu_row = rpool.tile([1, S], F32R, tag="r1")
        nc.scalar.activation(mu_row[:], ps1[:], AF.Identity, scale=1.0 / H)
        ps_mu = lnps.tile([128, S], F32, tag="bc", bufs=1, name="ln_mu_b")
        nc.tensor.matmul(ps_mu[:], ones_row[:], mu_row[:], start=True, stop=True)
        mu_b = tpool.tile([128, S], F32, tag="mub", bufs=2)
        nc.vector.tensor_copy(mu_b[:], ps_mu[:])
        # keep-warm: rewrite the same broadcast a few times so the PE stays
        # busy (and out of the cold p-state) while the rstd row chain runs
        for _ in range(8):
            nc.tensor.matmul(ps_mu[:], ones_row[:], mu_row[:],
                             start=True, stop=True, skip_group_check=True)

        ex2 = rpool.tile([1, S], F32, tag="r1")
        nc.scalar.activation(ex2[:], ps2[:], AF.Identity, scale=1.0 / H)
        musq = rpool.tile([1, S], F32, tag="r1")
        nc.scalar.activation(musq[:], mu_row[:].bitcast(F32), AF.Square)
        var = rpool.tile([1, S], F32, tag="r1")
        nc.vector.tensor_tensor(var[:], ex2[:], musq[:], op=OP.subtract)
        sd = rpool.tile([1, S], F32, tag="r1")
        nc.scalar.activation(sd[:], var[:], AF.Sqrt, bias=eps_t[:])
        rstd_row = rpool.tile([1, S], F32R, tag="r1")
        with nc.allow_low_precision("f32r rstd"):
            nc.vector.reciprocal(rstd_row[:], sd[:])
        ps_rstd = lnps.tile([128, S], F32, tag="bc2", bufs=1, name="ln_rstd_b")
        nc.tensor.matmul(ps_rstd[:], ones_row[:], rstd_row[:], start=True, stop=True)
        rstd_b = tpool.tile([128, S], F32, tag="rsb", bufs=2)
        nc.vector.tensor_copy(rstd_b[:], ps_rstd[:])

        out = spool.tile([128, KC, S], out_dtype, tag="s6")
        for k in range(KC):
            tmp = tpool.tile([128, S], F32, tag="lntmp", bufs=3)
            nc.vector.tensor_tensor(tmp[:], x[:, k].bitcast(F32), mu_b[:],
                                    op=OP.subtract)
            nc.vector.tensor_tensor(tmp[:], tmp[:], rstd_b[:], op=OP.mult)
            nc.scalar.activation(out[:, k], tmp[:], AF.Identity,
                                 scale=scale_col[:, k:k + 1],
                                 bias=bias_col[:, k:k + 1])
    return out


def _encoder_layer(nc, tc, l, x,
                   spool, epool, wcpool, wrpool, tpool, gpool, rpool, ppool,
                   v_aug, zmask, ones_col, ones_row, eps_t,
                   wq_d, bq_d, wk_d, bk_d, wv_d, bv_d, wo_d, bo_d,
                   l1s_d, l1b_d, w1_d, b1_d, w2_d, b2_d, l2s_d, l2b_d,
                   last=False):
    # ---- per-layer params -------------------------------------------------
    def col6(dram):
        t = ppool.tile([128, KC], F32, tag="pc6", bufs=18)
        nc.sync.dma_start(t[:], dram[l].rearrange("(k p) -> p k", p=128))
        return t

    bq_c, bk_c, bo_c, b2_c = col6(bq_d), col6(bk_d), col6(bo_d), col6(b2_d)
    l1s_c, l1b_c, l2s_c, l2b_c = col6(l1s_d), col6(l1b_d), col6(l2s_d), col6(l2b_d)
    b1_c = ppool.tile([128, FC], F32, tag="pc24", bufs=3)
    nc.sync.dma_start(b1_c[:], b1_d[l].rearrange("(k p) -> p k", p=128))

    bv_row = rpool.tile([1, H], F32R, tag="rh", bufs=1)
    nc.sync.dma_start(bv_row[:], bv_d[l:l + 1, :].bitcast(F32R))

    # ---- QKV --------------------------------------------------------------
    qT = spool.tile([128, KC, S], F32R, tag="s6")
    kT = spool.tile([128, KC, S], F32R, tag="s6")

    with tc.tile_pool(name="qkvps", bufs=3, space="PSUM") as qkps:
        # bv broadcast to [128, H] (token-major bias for V)
        bv_b = tpool.tile([128, H], F32, tag="tbh", bufs=1)
        for n0, nsz in ((0, 512), (512, 256)):
            psb = qkps.tile([128, 512], F32, tag="mm", name=f"bvb{n0}")
            nc.tensor.matmul(psb[:, :nsz], ones_row[:], bv_row[:, n0:n0 + nsz],
                             start=True, stop=True)
            nc.vector.tensor_copy(bv_b[:, n0:n0 + nsz], psb[:, :nsz])

        for dst, w_d, b_c in ((qT, wq_d, bq_c), (kT, wk_d, bk_c)):
            for m in range(KC):
                wt = wcpool.tile([128, KC, 128], F32R, tag="wc")
                nc.sync.dma_start(
                    wt[:],
                    w_d[l, :, 128 * m:128 * (m + 1)]
                    .rearrange("(ko ki) f -> ki ko f", ki=128).bitcast(F32R),
                )
                ps = qkps.tile([128, 512], F32, tag="mm", name=f"qk{m}")
                for k in range(KC):
                    nc.tensor.matmul(ps[:], wt[:, k], x[:, k],
                                     start=(k == 0), stop=(k == KC - 1))
                nc.scalar.activation(dst[:, m], ps[:], AF.Identity,
                                      bias=b_c[:, m:m + 1])

        # V token-major: lhsT = x chunk (feats x toks), rhs = wv row-chunk
        wv_t = []
        for k in range(KC):
            wvt = wrpool.tile([128, H], F32R, tag="wvr", bufs=KC, name=f"wv{k}")
            nc.sync.dma_start(wvt[:], wv_d[l, 128 * k:128 * (k + 1), :].bitcast(F32R))
            wv_t.append(wvt)
        for t in range(TC):
            for n0, nsz in ((0, 512), (512, 256)):
                ps = qkps.tile([128, 512], F32, tag="mm", name=f"v{t}{n0}")
                for k in range(KC):
                    nc.tensor.matmul(
                        ps[:, :nsz],
                        x[:, k, 128 * t:128 * (t + 1)],
                        wv_t[k][:, n0:n0 + nsz],
                        start=(k == 0), stop=(k == KC - 1),
                    )
                nh0, nh1 = n0 // 64, (n0 + nsz) // 64
                nc.vector.tensor_tensor(
                    v_aug[:, t].rearrange("p (h c) -> p h c", c=65)[:, nh0:nh1, :64],
                    ps[:, :nsz].rearrange("p (h c) -> p h c", c=64),
                    bv_b[:, n0:n0 + nsz].rearrange("p (h c) -> p h c", c=64),
                    op=OP.add,
                )
            nc.vector.tensor_scalar(v_aug[:, t], v_aug[:, t].bitcast(F32),
                                    zmask[:, t:t + 1], None, OP.mult)

    # ---- attention --------------------------------------------------------
    ctxT = spool.tile([128, KC, S], F32R, tag="s6")
    with tc.tile_pool(name="attps", bufs=1, space="PSUM") as atps:
        for hc in range(KC):
            pair = []
            for h in (2 * hc, 2 * hc + 1):
                fo = 64 * (h % 2)
                expT = epool.tile([128, TC, S], F32R, tag="exp")
                for half in range(2):
                    psl = atps.tile([128, 2, S], F32, tag="lg", bufs=2,
                                    name=f"lg{h}_{half}")
                    for i in range(2):
                        kt = 2 * half + i
                        nc.tensor.matmul(
                            psl[:, i],
                            kT[fo:fo + 64, hc, 128 * kt:128 * (kt + 1)],
                            qT[fo:fo + 64, hc, :],
                            start=True, stop=True,
                        )
                    nc.scalar.activation(expT[:, 2 * half:2 * half + 2], psl[:],
                                         AF.Exp, scale=0.125)
                psc = atps.tile([65, S], F32, tag="cx", bufs=2, name=f"cx{h}")
                for t in range(TC):
                    nc.tensor.matmul(psc[:], v_aug[:, t, 65 * h:65 * h + 65],
                                     expT[:, t], start=(t == 0), stop=(t == TC - 1))
                rec_row = rpool.tile([1, S], F32R, tag="r1")
                with nc.allow_low_precision("f32r recip"):
                    nc.vector.reciprocal(rec_row[:], psc[64:65, :])
                pair.append((h, fo, psc, rec_row))
            for h, fo, psc, rec_row in pair:
                psb = atps.tile([64, S], F32, tag="bc", bufs=2, name=f"bc{h}")
                nc.tensor.matmul(psb[:], ones_row[:, :64], rec_row[:],
                                 start=True, stop=True)
                rec_sb = tpool.tile([64, S], F32, tag="rec", bufs=2)
                nc.scalar.activation(rec_sb[:], psb[:], AF.Identity)
                nc.vector.tensor_tensor(ctxT[fo:fo + 64, hc, :], psc[:64, :],
                                        rec_sb[:], op=OP.mult)

    # ---- output projection + residual + LN1 -------------------------------
    s1 = spool.tile([128, KC, S], F32R, tag="s6")
    with tc.tile_pool(name="ops", bufs=3, space="PSUM") as ops:
        for m in range(KC):
            wt = wcpool.tile([128, KC, 128], F32R, tag="wc")
            nc.sync.dma_start(
                wt[:],
                wo_d[l, :, 128 * m:128 * (m + 1)]
                .rearrange("(ko ki) f -> ki ko f", ki=128).bitcast(F32R),
            )
            ps = ops.tile([128, 512], F32, tag="mm", name=f"o{m}")
            for k in range(KC):
                nc.tensor.matmul(ps[:], wt[:, k], ctxT[:, k],
                                 start=(k == 0), stop=(k == KC - 1))
            ot = tpool.tile([128, S], F32, tag="resid", bufs=3)
            nc.scalar.activation(ot[:], ps[:], AF.Identity, bias=bo_c[:, m:m + 1])
            nc.vector.tensor_tensor(s1[:, m], ot[:], x[:, m].bitcast(F32), op=OP.add)
        y = _layer_norm(nc, tc, spool, tpool, rpool, s1, l1s_c, l1b_c,
                        ones_col, ones_row, eps_t, F32R, psum_pool=ops)

    # ---- FFN --------------------------------------------------------------
    s2 = spool.tile([128, KC, S], F32R, tag="s6")
    with tc.tile_pool(name="ffps", bufs=1, space="PSUM") as fps:
        acc = [fps.tile([128, S], F32, tag=f"f2_{m}", name=f"f2_{m}")
               for m in range(KC)]
        for j in range(FC):
            w1t = wcpool.tile([128, KC, 128], F32R, tag="wc")
            nc.sync.dma_start(
                w1t[:],
                w1_d[l, :, 128 * j:128 * (j + 1)]
                .rearrange("(ko ki) f -> ki ko f", ki=128).bitcast(F32R),
            )
            psg = fps.tile([128, S], F32, tag="f1", bufs=2, name=f"g{j}")
            for k in range(KC):
                nc.tensor.matmul(psg[:], w1t[:, k], y[:, k],
                                 start=(k == 0), stop=(k == KC - 1))
            g = gpool.tile([128, S], F32R, tag="g512")
            nc.scalar.activation(g[:], psg[:], AF.Gelu, bias=b1_c[:, j:j + 1])

            w2t = wrpool.tile([128, H], F32R, tag="wr")
            nc.sync.dma_start(w2t[:], w2_d[l, 128 * j:128 * (j + 1), :].bitcast(F32R))
            for m in range(KC):
                nc.tensor.matmul(acc[m][:], w2t[:, 128 * m:128 * (m + 1)], g[:],
                                 start=(j == 0), stop=(j == FC - 1),
                                 skip_group_check=True)
        for m in range(KC):
            ot = tpool.tile([128, S], F32, tag="resid", bufs=3)
            nc.scalar.activation(ot[:], acc[m][:], AF.Identity, bias=b2_c[:, m:m + 1])
            nc.vector.tensor_tensor(s2[:, m], ot[:], y[:, m].bitcast(F32), op=OP.add)

    return _layer_norm(nc, tc, spool, tpool, rpool, s2, l2s_c, l2b_c,
                       ones_col, ones_row, eps_t, F32 if last else F32R)


# --- host-side entry --------------------------------------------------------

_nc_cache = {}


def _get_nc(n_layers=L):
    if n_layers not in _nc_cache:
        _install_waitfix()
        _nc_cache[n_layers] = build_nc(n_layers)
    return _nc_cache[n_layers]


def kernel(**inputs):
    from concourse import bass_utils

    nc = _get_nc(L)
    in_maps = []
    for b in range(N_CORES):
        m = {
            "input_ids": np.ascontiguousarray(inputs["input_ids"][b]),
            "type_ids": np.ascontiguousarray(inputs["type_ids"][b]),
        }
        for k in ("word_emb", "pos_emb", "type_emb", "emb_ln_scale", "emb_ln_bias",
                  "wq", "bq", "wk", "bk", "wv", "bv", "wo", "bo",
                  "ln1_scale", "ln1_bias", "w1", "b1", "w2", "b2",
                  "ln2_scale", "ln2_bias"):
            m[k] = np.asarray(inputs[k])
        in_maps.append(m)
    res = bass_utils.run_bass_kernel_spmd(nc, in_maps, core_ids=list(range(N_CORES)))
    return np.stack([r["out"] for r in res.results], axis=0)

